# revision 1
# baseline (speedup 1.0000x reference)
"""NeighborhoodAttention1D kernel for 8 Trainium2 NeuronCores.

Sequence-parallel sharding: each of the 8 cores handles 1024 consecutive
query positions (batch b = core//4, chunk j = core%4), with a 16-token
K/V halo on each side (zero-padded at batch edges; boundary-clamped
windows never read the padding).

Per-core pipeline (all on-chip after the initial loads):
  phase 1: qkv^T = W_qkv^T-style matmuls producing q^T,k^T in
           [feature, token] layout and V in natural [token, feature]
           layout (bias for V folded in via a ones-row matmul).
  phase 2: neighborhood attention per (head, 128-key chunk): scores are
           computed transposed (S^T = K^T.T @ Q^T blocks), exp on the
           scalar engine, multiplicative 0/1 band mask on the vector
           engine, then the masked-exp block is used as the stationary
           operand of two matmuls: attn@V (natural output) and the
           softmax denominator (ones column).  Normalization happens on
           eviction with a per-partition reciprocal.
  phase 3: PE transposes of the [query, 512] attention output feed the
           output projection (bias via ones-row matmul).

Compute is bf16 on the PE (fp32 matmul is 4 cycles/row on trn2; bf16 is
1), accumulation fp32 in PSUM.
"""

import time
from contextlib import ExitStack

import ml_dtypes
import numpy as np

import concourse.bass as bass
import concourse.tile as tile
from concourse import bacc, mybir
from concourse.bass_utils import run_bass_kernel_spmd
from concourse.masks import make_identity

B, L, DIM = 2, 4096, 512
HEADS, KS = 8, 33
HD = DIM // HEADS          # 64
SCALE = HD ** -0.5
NCORES = 8
CHUNK = 1024               # queries per core
HALO = KS // 2             # 16
TOK = CHUNK + 2 * HALO     # 1056 local tokens per core
NQT = CHUNK // 128         # 8 query tiles
NKC = 9                    # key chunks: 8 full + 1 of 32 rows

BF = mybir.dt.bfloat16
F32 = mybir.dt.float32
NPBF = ml_dtypes.bfloat16

_cache = {}


def _block_geom(c):
    """(kw, q0, qn) for key-chunk block c: key rows [128c, 128c+kw),
    query token columns [q0, q0+qn)."""
    if c == 0:
        return 128, 16, 128
    if c == NKC - 1:
        return 32, 1008, 32
    return 128, 128 * c - 16, 160


def _tile_mask_geom(t):
    """Key rows for query tile t: main = chunk t (128 keys), corner =
    first 32 keys of chunk t+1 (only queries 96:128 of the tile reach
    them; the mask zeroes the rest)."""
    return 16 + 128 * t  # first query token column


def _build_bass(dbg=False, niter=1, parts=(1, 2, 3), loads=True):
    nc = bacc.Bacc("TRN2", target_bir_lowering=False, debug=False,
                   num_devices=NCORES)

    xT = nc.dram_tensor("xT", [4, 128, TOK], BF, kind="ExternalInput").ap()
    wqkvT = nc.dram_tensor("wqkvT", [4, 128, 3 * DIM], BF,
                           kind="ExternalInput").ap()
    wprojT = nc.dram_tensor("wprojT", [4, 128, DIM], BF,
                            kind="ExternalInput").ap()
    bqk = nc.dram_tensor("bqk", [128, 8], F32, kind="ExternalInput").ap()
    bv = nc.dram_tensor("bv", [1, DIM], BF, kind="ExternalInput").ap()
    bp = nc.dram_tensor("bp", [1, DIM], BF, kind="ExternalInput").ap()
    masks = nc.dram_tensor("masks", [NQT, 128, 256], BF,
                           kind="ExternalInput").ap()
    out = nc.dram_tensor("out", [CHUNK, DIM], F32, kind="ExternalOutput").ap()
    itercheck = None
    if niter > 1:
        itercheck = nc.dram_tensor("itercheck", [1, 8], F32,
                                   kind="ExternalOutput").ap()
    if dbg:
        d_qkT = nc.dram_tensor("d_qkT", [8, 128, TOK], BF,
                               kind="ExternalOutput").ap()
        d_vnat = nc.dram_tensor("d_vnat", [NKC, 128, DIM], BF,
                                kind="ExternalOutput").ap()
        d_ao = nc.dram_tensor("d_ao", [NQT, 128, DIM], BF,
                              kind="ExternalOutput").ap()
        d_pS = nc.dram_tensor("d_pS", [128, 256], F32,
                              kind="ExternalOutput").ap()
        d_msk = nc.dram_tensor("d_msk", [128, 256], F32,
                               kind="ExternalOutput").ap()
        d_po = nc.dram_tensor("d_po", [128, 65], F32,
                              kind="ExternalOutput").ap()

    with tile.TileContext(nc) as tc, ExitStack() as ctx:
        sb = ctx.enter_context(tc.tile_pool(name="sb", bufs=1))
        ps = ctx.enter_context(tc.tile_pool(name="ps", bufs=1, space="PSUM"))

        # ---- static SBUF ----
        xT_sb = [sb.tile([128, TOK], BF, tag=f"xT{i}", name=f"xT{i}") for i in range(4)]
        wq_sb = [sb.tile([128, 3 * DIM], BF, tag=f"wq{i}", name=f"wq{i}") for i in range(4)]
        wp_sb = [sb.tile([128, DIM], BF, tag=f"wp{i}", name=f"wp{i}") for i in range(4)]
        bqk_sb = sb.tile([128, 8], F32, tag="bqk", name="bqk")
        bv_sb = sb.tile([1, DIM], BF, tag="bv", name="bv")
        bp_sb = sb.tile([1, DIM], BF, tag="bp", name="bp")
        mask_sb = [sb.tile([128, 256], BF, tag=f"mask{t}", name=f"mask{t}")
                   for t in range(NQT)]
        ones_row = sb.tile([1, 128], BF, tag="ones_row", name="ones_row")
        ones_col = sb.tile([128, 1], BF, tag="ones_col", name="ones_col")
        ident = sb.tile([128, 128], BF, tag="ident", name="ident")

        nc.vector.memset(ones_row[:], 1.0)
        nc.vector.memset(ones_col[:], 1.0)
        make_identity(nc, ident[:])
        if itercheck is not None:
            ic_sb = sb.tile([1, 8], F32, tag="ic", name="ic")
            nc.vector.memset(ic_sb[:], float(niter))

        def emit_loads():
            for i in range(4):
                nc.sync.dma_start(xT_sb[i][:], xT[i])
                nc.sync.dma_start(wq_sb[i][:], wqkvT[i])
                nc.sync.dma_start(wp_sb[i][:], wprojT[i])
            nc.sync.dma_start(bqk_sb[:], bqk[:])
            nc.sync.dma_start(bv_sb[:], bv[:])
            nc.sync.dma_start(bp_sb[:], bp[:])
            for t in range(NQT):
                nc.sync.dma_start(mask_sb[t][:], masks[t])

        qkT_sb = [sb.tile([128, TOK], BF, tag=f"qkT{oc}", name=f"qkT{oc}") for oc in range(8)]
        vnat_sb = [sb.tile([128, DIM], BF, tag=f"vnat{t}", name=f"vnat{t}") for t in range(NKC)]
        ao_sb = [sb.tile([128, DIM], BF, tag=f"ao{t}", name=f"ao{t}") for t in range(NQT)]

        work = ctx.enter_context(tc.tile_pool(name="work", bufs=1))

        # ---- phase 1: q^T / k^T ([feature, token]) ----
        TT = [(0, 512), (512, 512), (1024, TOK - 1024)]
        # head h uses q chunk h//2 and k chunk 4+h//2; emit in an order that
        # unblocks head 0 earliest.
        oc_order = [0, 4, 1, 5, 2, 6, 3, 7]

        def emit_qk(oc):
            pt = [ps.tile([128, 512], F32, tag="qkv", name="qkv", bufs=2) for _ in TT]
            for ic in range(4):
                for tt, (t0, tw) in enumerate(TT):
                    nc.tensor.matmul(
                        pt[tt][:, :tw],
                        lhsT=wq_sb[ic][:, oc * 128:(oc + 1) * 128],
                        rhs=xT_sb[ic][:, t0:t0 + tw],
                        start=(ic == 0), stop=(ic == 3),
                    )
            for tt, (t0, tw) in enumerate(TT):
                nc.scalar.activation(
                    out=qkT_sb[oc][:, t0:t0 + tw], in_=pt[tt][:, :tw],
                    func=mybir.ActivationFunctionType.Identity,
                    bias=bqk_sb[:, oc:oc + 1], scale=1.0,
                )

        def emit_vnat(vt):
            pw = 128 if vt < NKC - 1 else TOK - 128 * (NKC - 1)
            p = ps.tile([128, 512], F32, tag="qkv", name="qkv", bufs=2)
            for ic in range(4):
                nc.tensor.matmul(
                    p[:pw, :],
                    lhsT=xT_sb[ic][:, vt * 128:vt * 128 + pw],
                    rhs=wq_sb[ic][:, 2 * DIM:3 * DIM],
                    start=(ic == 0), stop=False,
                )
            nc.tensor.matmul(
                p[:pw, :], lhsT=ones_row[:1, :pw], rhs=bv_sb[:1, :],
                start=False, stop=True,
            )
            nc.vector.tensor_copy(vnat_sb[vt][:pw, :], p[:pw, :])

        for _it in range(niter):
          if itercheck is not None:
            nc.sync.dma_start(itercheck[:], ic_sb[:])
          if loads:
            emit_loads()
          if 1 in parts:
            emit_qk(0)
            emit_qk(4)
            for vt in range(NKC):
              emit_vnat(vt)
            for oc in oc_order[2:]:
              emit_qk(oc)

          # ---- phase 2: attention ----
          for h in (range(HEADS) if 2 in parts else []):
              qT = qkT_sb[h // 2][(h % 2) * 64:(h % 2) * 64 + 64, :]
              kT = qkT_sb[4 + h // 2][(h % 2) * 64:(h % 2) * 64 + 64, :]
              for t in range(NQT):
                  q0 = 16 + 128 * t
                  k0 = 128 * t
                  pS = ps.tile([128, 256], F32, tag="S", name="S", bufs=3)
                  nc.tensor.matmul(
                      pS[:, 0:128], lhsT=kT[:, k0:k0 + 128],
                      rhs=qT[:, q0:q0 + 128], start=True, stop=True,
                  )
                  nc.tensor.matmul(
                      pS[0:32, 128:256], lhsT=kT[:, k0 + 128:k0 + 160],
                      rhs=qT[:, q0:q0 + 128], start=True, stop=True,
                      skip_group_check=True,
                  )
                  # one exp + one masked-mul over the whole [128,256] block;
                  # rows 32:128 of the corner half are garbage psum (exp may
                  # produce inf, mask 0 -> NaN) but are never read downstream.
                  msk = work.tile([128, 256], BF, tag="msk", name="msk", bufs=3)
                  expS = work.tile([128, 256], BF, tag="expS", name="expS",
                                   bufs=3)
                  nc.scalar.activation(out=expS[:], in_=pS[:],
                                       func=mybir.ActivationFunctionType.Exp)
                  nc.vector.tensor_mul(msk[:], expS[:], mask_sb[t][:])

                  po = ps.tile([128, 65], F32, tag="po", name="po", bufs=3)
                  nc.tensor.matmul(
                      po[:, 0:64], lhsT=msk[:, 0:128],
                      rhs=vnat_sb[t][:, h * 64:h * 64 + 64],
                      start=True, stop=False,
                  )
                  # NOTE: start=True clears has_written for the whole PSUM
                  # bank, so only the first matmul into this tile may set it;
                  # later writes to untouched elements initialize them anyway.
                  nc.tensor.matmul(
                      po[:, 64:65], lhsT=msk[:, 0:128], rhs=ones_col[:, :1],
                      start=False, stop=False, skip_group_check=True,
                  )
                  nc.tensor.matmul(
                      po[:, 0:64], lhsT=msk[0:32, 128:256],
                      rhs=vnat_sb[t + 1][0:32, h * 64:h * 64 + 64],
                      start=False, stop=True, skip_group_check=True,
                  )
                  nc.tensor.matmul(
                      po[:, 64:65], lhsT=msk[0:32, 128:256],
                      rhs=ones_col[0:32, :1],
                      start=False, stop=True, skip_group_check=True,
                  )
                  r = work.tile([128, 1], F32, tag="r", name="r", bufs=3)
                  nc.vector.reciprocal(r[:], po[:, 64:65])
                  nc.vector.tensor_scalar_mul(
                      ao_sb[t][:, h * 64:h * 64 + 64], po[:, 0:64], r[:]
                  )
                  if dbg and h == 0 and t == 1:
                      stg1 = sb.tile([128, 256], F32, name="stg1")
                      nc.vector.tensor_copy(stg1[:], pS[:])
                      nc.sync.dma_start(d_pS[:], stg1[:])
                      stg2 = sb.tile([128, 256], F32, name="stg2")
                      nc.vector.tensor_copy(stg2[:], msk[:])
                      nc.sync.dma_start(d_msk[:], stg2[:])
                      stg3 = sb.tile([128, 65], F32, name="stg3")
                      nc.vector.tensor_copy(stg3[:], po[:])
                      nc.sync.dma_start(d_po[:], stg3[:])

          # ---- phase 3: output projection ----
          for t in (range(NQT) if 3 in parts else []):
              aoT = []
              for icc in range(4):
                  pT = ps.tile([128, 128], BF, tag="qkv", name="pT", bufs=2)
                  nc.tensor.transpose(
                      pT[:], ao_sb[t][:, icc * 128:(icc + 1) * 128], ident[:]
                  )
                  aT = work.tile([128, 128], BF, tag="aoT", name="aoT", bufs=8)
                  nc.vector.tensor_copy(aT[:], pT[:])
                  aoT.append(aT)
              pout = ps.tile([128, 512], F32, tag="qkv", name="pout", bufs=2)
              for icc in range(4):
                  nc.tensor.matmul(pout[:], lhsT=aoT[icc][:], rhs=wp_sb[icc][:],
                                   start=(icc == 0), stop=False)
              nc.tensor.matmul(pout[:], lhsT=ones_row[:1, :128], rhs=bp_sb[:1, :],
                               start=False, stop=True)
              osb = work.tile([128, 512], F32, tag="osb", name="osb", bufs=3)
              nc.scalar.copy(osb[:], pout[:])
              nc.sync.dma_start(out[t * 128:(t + 1) * 128, :], osb[:])

          if dbg:
              for oc in range(8):
                  nc.sync.dma_start(d_qkT[oc], qkT_sb[oc][:])
              for vt in range(NKC):
                  nc.sync.dma_start(d_vnat[vt], vnat_sb[vt][:])
              for t in range(NQT):
                  nc.sync.dma_start(d_ao[t], ao_sb[t][:])

    nc.finalize()
    return nc


def _host_prep(x, w_qkv, b_qkv, w_proj, b_proj):
    """Build the 8 per-core input maps."""
    x = np.asarray(x, np.float32)
    w_qkv = np.asarray(w_qkv, np.float32)
    b_qkv = np.asarray(b_qkv, np.float32)
    w_proj = np.asarray(w_proj, np.float32)
    b_proj = np.asarray(b_proj, np.float32)

    wt = w_qkv.T.copy()                      # [512, 1536]
    wt[:, :DIM] *= SCALE                     # fold attention scale into W_q
    bq = b_qkv.copy()
    bq[:DIM] *= SCALE
    wqkvT = wt.reshape(DIM, 3 * DIM).astype(NPBF).reshape(4, 128, 3 * DIM)
    wprojT = w_proj.T.copy().astype(NPBF).reshape(4, 128, DIM)
    bqk = bq[:1024].reshape(8, 128).T.copy().astype(np.float32)
    bv = bq[2 * DIM:3 * DIM].reshape(1, DIM).astype(NPBF)
    bpj = b_proj.reshape(1, DIM).astype(NPBF)

    starts = np.clip(np.arange(L) - HALO, 0, L - KS)   # global window starts

    in_maps = []
    for core in range(NCORES):
        b, j = divmod(core, 4)
        base = j * CHUNK - HALO
        lo, hi = max(0, base), min(L, base + TOK)
        xs = np.zeros((TOK, DIM), np.float32)
        xs[lo - base:hi - base] = x[b, lo:hi]
        xTc = xs.T.copy().astype(NPBF).reshape(4, 128, TOK)

        # mask[t]: [key within chunk, query] — cols 0:128 vs chunk t keys,
        # cols 128:256 vs the first 32 keys of chunk t+1 (rows 32:128 zero).
        mk = np.zeros((NQT, 128, 256), np.float32)
        for t in range(NQT):
            kg = base + 128 * t + np.arange(160)
            qg = base + 16 + 128 * t + np.arange(128)
            ws = starts[qg]
            band = ((kg[:, None] >= ws[None, :])
                    & (kg[:, None] <= ws[None, :] + KS - 1))   # [160, 128]
            mk[t, :, 0:128] = band[0:128]
            mk[t, 0:32, 128:256] = band[128:160]
        in_maps.append({
            "xT": xTc, "wqkvT": wqkvT, "wprojT": wprojT,
            "bqk": bqk, "bv": bv, "bp": bpj,
            "masks": mk.astype(NPBF),
        })
    return in_maps


def kernel(x, w_qkv, b_qkv, w_proj, b_proj):
    if "nc" not in _cache:
        _cache["nc"] = _build_bass()
    nc = _cache["nc"]
    in_maps = _host_prep(x, w_qkv, b_qkv, w_proj, b_proj)
    res = run_bass_kernel_spmd(nc, in_maps, core_ids=list(range(NCORES)))
    full = np.empty((B, L, DIM), np.float32)
    for core in range(NCORES):
        b, j = divmod(core, 4)
        full[b, j * CHUNK:(j + 1) * CHUNK] = res.results[core]["out"]
    return full



# revision 31
# speedup vs baseline: 192.0381x; 192.0381x over previous
"""NeighborhoodAttention1D kernel for 8 Trainium2 NeuronCores.

Sequence-parallel sharding: each of the 8 cores handles 1024 consecutive
query positions (batch b = core//4, chunk j = core%4), with a 16-token
K/V halo on each side (zero-padded at batch edges; boundary-clamped
windows never read the padding).

Per-core pipeline (all on-chip after the initial loads):
  phase 1: q^T,k^T in [feature, token] layout (bias folded in on ACT
           eviction), V in natural [token, feature] layout with the
           V-bias folded via a ones-row matmul; V is stored strided as
           [v_h | 1] blocks of 65 columns per head so the softmax
           denominator falls out of the attn@V matmul for free.
  phase 2: units of (query-tile t, head-group of 4): 8 score matmuls
           into one 2-bank PSUM supertile, one batched exp (ACT), one
           batched 0/1 band-mask multiply (any-engine, mask broadcast
           across the 4 heads), 8 attn@V matmuls into a per-unit
           [128, 4*65] PSUM tile, then a batched reciprocal +
           broadcast-multiply normalization straight into the
           attention-output tile.
  phase 3: PE transposes of [query, 512] attention output feed the
           output projection; the projection bias is added on the host.

Compute is bf16 on the PE, fp32 accumulation in PSUM, bf16 output
(the host adds b_proj and casts to fp32).
"""

from contextlib import ExitStack

import ml_dtypes
import numpy as np

import concourse.bass as bass
import concourse.tile as tile
from concourse import bacc, mybir
from concourse.bass_utils import run_bass_kernel_spmd
from concourse.masks import make_identity

B, L, DIM = 2, 4096, 512
HEADS, KS = 8, 33
HD = DIM // HEADS          # 64
SCALE = HD ** -0.5
NCORES = 8
CHUNK = 1024               # queries per core
HALO = KS // 2             # 16
TOK = CHUNK + 2 * HALO     # 1056 local tokens per core
NQT = CHUNK // 128         # 8 query tiles
NVT = 9                    # v tiles: 8 full + 1 of 32 rows

BF = mybir.dt.bfloat16
F32 = mybir.dt.float32
NPBF = ml_dtypes.bfloat16

_cache = {}


def _build_bass(niter=1, dbg=False):
    nc = bacc.Bacc("TRN2", target_bir_lowering=False, debug=False,
                   num_devices=NCORES)

    xT = nc.dram_tensor("xT", [128, 4 * TOK], BF, kind="ExternalInput").ap()
    wqkvT = nc.dram_tensor("wqkvT", [128, 4 * 1024], BF,
                           kind="ExternalInput").ap()
    wvT = nc.dram_tensor("wvT", [128, 4 * DIM], BF, kind="ExternalInput").ap()
    wprojT = nc.dram_tensor("wprojT", [128, 4 * DIM], BF,
                            kind="ExternalInput").ap()
    bqk = nc.dram_tensor("bqk", [128, 8], F32, kind="ExternalInput").ap()
    masks = nc.dram_tensor("masks", [128, 3 * 256], BF,
                           kind="ExternalInput").ap()
    out = nc.dram_tensor("out", [CHUNK, DIM], BF, kind="ExternalOutput").ap()
    if dbg:
        d_qkT = nc.dram_tensor("d_qkT", [8, 128, TOK], BF,
                               kind="ExternalOutput").ap()
        d_vnat = nc.dram_tensor("d_vnat", [NVT, 128, 520], BF,
                                kind="ExternalOutput").ap()
        d_msk = nc.dram_tensor("d_msk", [2, 128, 1024], BF,
                               kind="ExternalOutput").ap()
        d_po4 = nc.dram_tensor("d_po4", [2, 128, 260], F32,
                               kind="ExternalOutput").ap()
        d_ao = nc.dram_tensor("d_ao", [NQT, 128, DIM], BF,
                              kind="ExternalOutput").ap()

    with tile.TileContext(nc) as tc, ExitStack() as ctx:
        sb = ctx.enter_context(tc.tile_pool(name="sb", bufs=1))
        ps = ctx.enter_context(tc.tile_pool(name="ps", bufs=1, space="PSUM"))
        work = ctx.enter_context(tc.tile_pool(name="work", bufs=1))

        # ---- static SBUF ----
        xT_all = sb.tile([128, 4 * TOK], BF, tag="xT", name="xT_all")
        wqk_all = sb.tile([128, 4 * 1024], BF, tag="wqk", name="wqk_all")
        wv_all = sb.tile([128, 4 * DIM], BF, tag="wv", name="wv_all")
        wp_all = sb.tile([128, 4 * DIM], BF, tag="wp", name="wp_all")
        xT_sb = [xT_all[:, i * TOK:(i + 1) * TOK] for i in range(4)]
        wqk_sb = [wqk_all[:, i * 1024:(i + 1) * 1024] for i in range(4)]
        wv_sb = [wv_all[:, i * DIM:(i + 1) * DIM] for i in range(4)]
        wp_sb = [wp_all[:, i * DIM:(i + 1) * DIM] for i in range(4)]
        bqk_sb = sb.tile([128, 8], F32, tag="bqk", name="bqk")
        mask_sb = sb.tile([128, 3 * 256], BF, tag="mask", name="mask")
        mrep_sb = sb.tile([128, 3 * 1024], BF, tag="mrep", name="mrep")
        ident = sb.tile([128, 128], BF, tag="ident", name="ident")

        make_identity(nc, ident[:])

        # persistent per-iteration SBUF
        qkT_sb = [sb.tile([128, TOK], BF, tag=f"qkT{oc}", name=f"qkT{oc}")
                  for oc in range(8)]
        vnat_sb = [sb.tile([128, 8 * 65], BF, tag=f"vnat{t}", name=f"vnat{t}")
                   for t in range(NVT)]
        ao_sb = [sb.tile([128, DIM], BF, tag=f"ao{t}", name=f"ao{t}")
                 for t in range(NQT)]

        def emit_loads():
            nc.sync.dma_start(xT_all[:], xT[:])
            nc.sync.dma_start(wqk_all[:], wqkvT[:])
            nc.sync.dma_start(wv_all[:], wvT[:])
            nc.sync.dma_start(wp_all[:], wprojT[:])
            nc.sync.dma_start(bqk_sb[:], bqk[:])
            nc.sync.dma_start(mask_sb[:], masks[:])

        # ---- phase 1 ----
        def emit_q(oc):
            # q^T chunk oc: only the core's own 1024 queries, cols 16:1040.
            pq = ps.tile([128, 1024], F32, tag="big", name="pq", bufs=2)
            for ic in range(4):
                for s in range(2):
                    nc.tensor.matmul(
                        pq[:, s * 512:(s + 1) * 512],
                        lhsT=wqk_sb[ic][:, oc * 128:(oc + 1) * 128],
                        rhs=xT_sb[ic][:, 16 + s * 512:16 + (s + 1) * 512],
                        start=(ic == 0), stop=(ic == 3),
                    )
            nc.scalar.activation(
                out=qkT_sb[oc][:, 16:1040], in_=pq[:, :],
                func=mybir.ActivationFunctionType.Identity,
                bias=bqk_sb[:, oc:oc + 1], scale=1.0,
            )

        def emit_k(oc):
            pk = ps.tile([128, 1024], F32, tag="big", name="pk", bufs=2)
            pkt = ps.tile([128, 32], F32, tag="po", name="pkt", bufs=2)
            for ic in range(4):
                for t0 in (0, 512):
                    nc.tensor.matmul(
                        pk[:, t0:t0 + 512],
                        lhsT=wqk_sb[ic][:, oc * 128:(oc + 1) * 128],
                        rhs=xT_sb[ic][:, t0:t0 + 512],
                        start=(ic == 0), stop=(ic == 3),
                    )
                nc.tensor.matmul(
                    pkt[:, :],
                    lhsT=wqk_sb[ic][:, oc * 128:(oc + 1) * 128],
                    rhs=xT_sb[ic][:, 1024:1056],
                    start=(ic == 0), stop=(ic == 3),
                )
            nc.scalar.activation(
                out=qkT_sb[oc][:, 0:1024], in_=pk[:, :],
                func=mybir.ActivationFunctionType.Identity,
                bias=bqk_sb[:, oc:oc + 1], scale=1.0,
            )
            nc.vector.tensor_scalar_add(qkT_sb[oc][:, 1024:1056], pkt[:, :],
                                        bqk_sb[:, oc:oc + 1])

        def emit_v(vt):
            pw = 128 if vt < NVT - 1 else TOK - 128 * (NVT - 1)
            pv = ps.tile([128, 512], F32, tag="big", name="pv", bufs=2)
            for ic in range(4):
                nc.tensor.matmul(
                    pv[:pw, :],
                    lhsT=xT_sb[ic][:, vt * 128:vt * 128 + pw],
                    rhs=wv_sb[ic][:, :],
                    start=(ic == 0), stop=(ic == 3),
                )
            v3 = vnat_sb[vt].rearrange("p (a b) -> p a b", b=65)
            nc.vector.tensor_copy(v3[:pw, :, 0:64],
                                  pv[:pw, :].rearrange("p (a b) -> p a b", a=8))

        for _it in range(niter):
            emit_loads()
            for vt in range(NVT):
                nc.any.memset(
                    vnat_sb[vt].rearrange("p (a b) -> p a b", b=65)[:, :, 64:65],
                    1.0)
            emit_q(0)
            emit_k(4)
            emit_q(1)
            emit_k(5)
            for vt in range(NVT):
                emit_v(vt)
            emit_q(2)
            emit_k(6)
            emit_q(3)
            emit_k(7)

            # ---- phase 2 + 3 ----
            for t in range(NQT):
                mslot = 0 if t == 0 else (2 if t == NQT - 1 else 1)
                for hg in range(2):
                    pS = ps.tile([128, 1024], F32, tag="big", name="pS", bufs=2)
                    for j in range(4):
                        h = 4 * hg + j
                        qT = qkT_sb[h // 2][(h % 2) * 64:(h % 2) * 64 + 64, :]
                        kT = qkT_sb[4 + h // 2][(h % 2) * 64:(h % 2) * 64 + 64, :]
                        c0 = 256 * (0, 2, 1, 3)[j]
                        q0 = 16 + 128 * t
                        k0 = 128 * t
                        nc.tensor.matmul(
                            pS[:, c0:c0 + 128], lhsT=kT[:, k0:k0 + 128],
                            rhs=qT[:, q0:q0 + 128], start=True, stop=True,
                            skip_group_check=(j > 0),
                        )
                        # corner: keys [k0+128, k0+160) reach only queries
                        # q_local >= 96 of this tile.
                        nc.tensor.matmul(
                            pS[0:32, c0 + 224:c0 + 256],
                            lhsT=kT[:, k0 + 128:k0 + 160],
                            rhs=qT[:, q0 + 96:q0 + 128], start=True, stop=True,
                            skip_group_check=True,
                        )
                    expS = work.tile([128, 1024], BF, tag="expS", name="expS",
                                     bufs=3)
                    nc.scalar.activation(out=expS[:], in_=pS[:],
                                         func=mybir.ActivationFunctionType.Exp)
                    mskS = work.tile([128, 1024], BF, tag="mskS", name="mskS",
                                     bufs=3)
                    m1 = mask_sb[:, mslot * 256:(mslot + 1) * 256]
                    eng = nc.vector if (t + hg) % 2 == 0 else nc.gpsimd
                    eng.tensor_mul(
                        mskS.rearrange("p (a b) -> p a b", a=4),
                        expS.rearrange("p (a b) -> p a b", a=4),
                        m1.unsqueeze(1).broadcast_to((128, 4, 256)),
                    )
                    po4 = ps.tile([128, 260], F32, tag="po", name="po4", bufs=2)
                    for j in range(4):
                        h = 4 * hg + j
                        c0 = 256 * (0, 2, 1, 3)[j]
                        nc.tensor.matmul(
                            po4[:, 65 * j:65 * j + 65],
                            lhsT=mskS[:, c0:c0 + 128],
                            rhs=vnat_sb[t][:, 65 * h:65 * h + 65],
                            start=True, stop=False, skip_group_check=(j > 0),
                        )
                        nc.tensor.matmul(
                            po4[96:128, 65 * j:65 * j + 65],
                            lhsT=mskS[0:32, c0 + 224:c0 + 256],
                            rhs=vnat_sb[t + 1][0:32, 65 * h:65 * h + 65],
                            start=False, stop=True, skip_group_check=True,
                            tile_position=(0, 96),
                        )
                    po3 = po4.rearrange("p (a b) -> p a b", b=65)
                    r4 = work.tile([128, 4], F32, tag="r4", name="r4", bufs=4)
                    nc.vector.reciprocal(r4.unsqueeze(2), po3[:, :, 64:65])
                    nc.vector.tensor_mul(
                        ao_sb[t][:, 256 * hg:256 * hg + 256]
                        .rearrange("p (a b) -> p a b", a=4),
                        po3[:, :, 0:64],
                        r4.unsqueeze(2).broadcast_to((128, 4, 64)),
                    )
                    if dbg and t == 0:
                        nc.sync.dma_start(d_msk[hg], mskS[:])
                        stg = sb.tile([128, 260], F32, name=f"stg{hg}")
                        nc.vector.tensor_copy(stg[:], po4[:])
                        nc.sync.dma_start(d_po4[hg], stg[:])

                # ---- phase 3 for tile t ----
                aoT = []
                for icc in range(4):
                    pT = ps.tile([128, 128], BF, tag="po", name="pT", bufs=2)
                    nc.tensor.transpose(
                        pT[:], ao_sb[t][:, icc * 128:(icc + 1) * 128], ident[:]
                    )
                    aT = work.tile([128, 128], BF, tag="aoT", name="aoT",
                                   bufs=8)
                    nc.vector.tensor_copy(aT[:], pT[:])
                    aoT.append(aT)
                pout = ps.tile([128, 512], F32, tag="pout", name="pout", bufs=2)
                for icc in range(4):
                    nc.tensor.matmul(pout[:], lhsT=aoT[icc][:],
                                     rhs=wp_sb[icc][:],
                                     start=(icc == 0), stop=(icc == 3))
                osb = work.tile([128, 512], BF, tag="osb", name="osb", bufs=3)
                nc.vector.tensor_copy(osb[:], pout[:])
                nc.sync.dma_start(out[t * 128:(t + 1) * 128, :], osb[:])

            if dbg:
                for oc in range(8):
                    nc.sync.dma_start(d_qkT[oc], qkT_sb[oc][:])
                for vt in range(NVT):
                    nc.sync.dma_start(d_vnat[vt], vnat_sb[vt][:])
                for t in range(NQT):
                    nc.sync.dma_start(d_ao[t], ao_sb[t][:])

    nc.finalize()
    return nc


def _host_prep(x, w_qkv, b_qkv, w_proj, b_proj):
    """Build the 8 per-core input maps."""
    x = np.asarray(x, np.float32)
    w_qkv = np.asarray(w_qkv, np.float32)
    b_qkv = np.asarray(b_qkv, np.float32)
    w_proj = np.asarray(w_proj, np.float32)

    wt = w_qkv.T.copy()                      # [512, 1536]
    wt[:, :DIM] *= SCALE                     # fold attention scale into W_q
    bq = b_qkv.copy()
    bq[:DIM] *= SCALE
    # [128, 4*1024]: per input-chunk ic, the q|k output columns.
    wqkT = np.concatenate(
        [wt.reshape(4, 128, 3 * DIM)[i, :, :1024] for i in range(4)], axis=1)
    wvT = np.concatenate(
        [wt.reshape(4, 128, 3 * DIM)[i, :, 2 * DIM:] for i in range(4)], axis=1)
    wprojT = np.concatenate(
        [w_proj.T.reshape(4, 128, DIM)[i] for i in range(4)], axis=1)
    bqk = bq[:1024].reshape(8, 128).T.copy().astype(np.float32)

    starts = np.clip(np.arange(L) - HALO, 0, L - KS)   # global window starts

    in_maps = []
    for core in range(NCORES):
        b, j = divmod(core, 4)
        base = j * CHUNK - HALO
        lo, hi = max(0, base), min(L, base + TOK)
        xs = np.zeros((TOK, DIM), np.float32)
        xs[lo - base:hi - base] = x[b, lo:hi]
        xTc = np.concatenate(
            [xs.T.reshape(4, 128, TOK)[i] for i in range(4)], axis=1)

        # mask slots: 0 -> tile t=0, 1 -> interior, 2 -> tile t=7.
        # layout per slot: [key within tile (128), 256] where cols 0:128
        # are the main queries vs main keys, cols 224:256 are queries
        # 96:128 vs the 32 corner keys (rows 0:32), cols 128:224 unused.
        mk = np.zeros((3, 128, 256), np.float32)
        for slot, t in ((0, 0), (1, 3), (2, NQT - 1)):
            kg = base + 128 * t + np.arange(160)
            qg = base + HALO + 128 * t + np.arange(128)
            ws = starts[np.clip(qg, 0, L - 1)]
            band = ((kg[:, None] >= ws[None, :])
                    & (kg[:, None] <= ws[None, :] + KS - 1))   # [160, 128]
            mk[slot, :, 0:128] = band[0:128]
            mk[slot, 0:32, 224:256] = band[128:160, 96:128]
        mkc = mk.transpose(1, 0, 2).reshape(128, 3 * 256)

        in_maps.append({
            "xT": xTc.astype(NPBF), "wqkvT": wqkT.astype(NPBF),
            "wvT": wvT.astype(NPBF), "wprojT": wprojT.astype(NPBF),
            "bqk": bqk, "masks": mkc.astype(NPBF),
        })
    return in_maps


def kernel(x, w_qkv, b_qkv, w_proj, b_proj):
    if "nc" not in _cache:
        _cache["nc"] = _build_bass()
    nc = _cache["nc"]
    in_maps = _host_prep(x, w_qkv, b_qkv, w_proj, b_proj)
    res = run_bass_kernel_spmd(nc, in_maps, core_ids=list(range(NCORES)))
    # V-bias passes through softmax normalization exactly, so it is folded
    # (with the projection bias) into one host-side constant.
    bq = np.asarray(b_qkv, np.float32)
    bias = (np.asarray(b_proj, np.float32)
            + bq[2 * DIM:3 * DIM] @ np.asarray(w_proj, np.float32).T)
    full = np.empty((B, L, DIM), np.float32)
    for core in range(NCORES):
        b, j = divmod(core, 4)
        full[b, j * CHUNK:(j + 1) * CHUNK] = (
            np.asarray(res.results[core]["out"], np.float32) + bias)
    return full
